# revision 23
# baseline (speedup 1.0000x reference)
"""Multi-head attention (B=8, N=1024, D=768, H=12) on 8 TRN2 NeuronCores.

Sharding: pure data parallel over batch — each core handles one batch
element; weights are replicated. No collectives.

Per-core dataflow (v2 — restructured from the 285us baseline for PE/ACT
overlap and to keep the PE HAM clock warm):

  1. qk^T tiles [128 feat, 1024 tok]: accumulated from block-resident
     wqk (6 DMA blocks of 6KB/partition) x xT in f32r; the qkv BIAS is
     folded into the PSUM->fp16 convert as a DVE tensor_scalar_add with
     a per-partition bias column (saves 2 PE matmuls per tile).
  2. scores^T per (pair, kt, head): ONE fp16 matmul [64K, 128M, 1024N]
     (fp16 moving operand max is 1024) into a [128, 1024] PSUM tile
     from a 2-buf pool -> exp(kt) on ACT overlaps scores(kt+1) on PE.
  3. softmax without max-subtraction (scores ~ N(0,1)); exp scale=1/8.
  4. attn@v: psum[0:65, qh*512] += [v_h | ones]^T @ attnT — 4 separate
     1-bank accumulators per round (i x qh) so pss can double-buffer.
  5. normalize: accumulators drained to SBUF immediately (frees PSUM),
     DVE reciprocal of the den row, gpsimd partition_broadcast (attn
     ucode library) to 64 rows, DVE multiply into f32r attn-out^T.
  6. proj: y = aoT^T @ w_proj + bias (K=1 matmul), DMA out per m-tile.

Rounds are per head-pair; attn@v trails scores by one kt so the PE
never waits on ACT. qk^T tiles for pair r+2 are produced as a burst
inside round r. v-projection tiles are woven into round 0.
"""

import sys

sys.path.insert(0, "/opt/trn_rl_repo")

import numpy as np

B, N, D, H, HD = 8, 1024, 768, 12, 64
F_QK = 2 * D  # 1536
SCALE = HD**-0.5
TOK_TILES = N // 128  # 8
D_SUB = D // 128  # 6
N_CORES = 8

_cached_nc = None


def _build():
    import concourse.tile as tile
    from concourse import bacc, library_config, mybir

    F32 = mybir.dt.float32
    F32R = mybir.dt.float32r
    FP16 = mybir.dt.float16
    BF16 = mybir.dt.bfloat16
    EXP = mybir.ActivationFunctionType.Exp
    MULT = mybir.AluOpType.mult

    nc = bacc.Bacc("TRN2", target_bir_lowering=False, debug=False)

    # x and the big weights ship as bf16 (halves HBM traffic; rel err
    # ~4e-3 vs the 2e-2 gate). Biases stay f32.
    xt_d = nc.dram_tensor("xt", [D, N], BF16, kind="ExternalInput").ap()
    wqkv_d = nc.dram_tensor("wqkv", [D, 3 * D], BF16, kind="ExternalInput").ap()
    bqkv_d = nc.dram_tensor("bqkv", [3 * D], F32R, kind="ExternalInput").ap()
    wproj_d = nc.dram_tensor("wproj", [D, D], BF16, kind="ExternalInput").ap()
    bproj_d = nc.dram_tensor("bproj", [D], F32R, kind="ExternalInput").ap()
    y_d = nc.dram_tensor("y", [N, D], F32, kind="ExternalOutput").ap()

    with tile.TileContext(nc) as tc:
        with (
            tc.tile_pool(name="singles", bufs=1) as singles,
            tc.tile_pool(name="qkT", bufs=12) as qkT_pool,
            tc.tile_pool(name="attnT", bufs=4) as attnT_pool,
            tc.tile_pool(name="aoU", bufs=3) as aoU_pool,
            tc.tile_pool(name="rrow", bufs=2) as rrow_pool,
            tc.tile_pool(name="rb", bufs=2) as rb_pool,
            tc.tile_pool(name="dram", bufs=2, space="DRAM") as dram_pool,
            tc.tile_pool(name="yout", bufs=3) as y_pool,
            tc.tile_pool(name="big", bufs=2, space="PSUM") as big,
            tc.tile_pool(name="acc", bufs=4, space="PSUM") as acc,
        ):
            # ---- resident SBUF tensors ----
            xT_sb = singles.tile([128, D_SUB, N], BF16)  # 12KB/part
            wqk_sb = singles.tile([128, D_SUB, F_QK], BF16)  # 18KB/part
            wv_sb = singles.tile([128, D_SUB, D], BF16)  # 9KB/part
            wproj_sb = singles.tile([128, D_SUB, D], BF16)  # 9KB/part
            v_sb = singles.tile([128, TOK_TILES, H * 65], FP16)  # 12.2KB/part
            bqk_col_r = singles.tile([128, 12], F32R)
            bqk_col = singles.tile([128, 12], F32)
            bv_sb = singles.tile([1, D], F32R)
            bp_sb = singles.tile([1, D], F32R)
            ones1 = singles.tile([1, 512], F32R)
            ones16 = singles.tile([128, 96], FP16)
            ones_f = singles.tile([128, 512], F32)

            # ---- setup DMAs, latency-critical order: x and wqk blocks
            # interleaved per d so qk matmuls can chase the stream ----
            xt_r = xt_d.rearrange("(o p) n -> p o n", p=128)
            wqk_r = wqkv_d[:, 0:F_QK].rearrange("(o p) f -> p o f", p=128)
            for d in range(D_SUB):
                nc.sync.dma_start(xT_sb[:, d, :], xt_r[:, d, :])
                nc.sync.dma_start(wqk_sb[:, d, :], wqk_r[:, d, :])
            # per-partition bias column layout: bqk_col[p, f] = bqkv[f*128+p]
            nc.sync.dma_start(
                bqk_col_r, bqkv_d[0:F_QK].rearrange("(f p) -> p f", p=128)
            )
            nc.sync.dma_start(bv_sb, bqkv_d[None, F_QK : 3 * D])
            nc.sync.dma_start(bp_sb, bproj_d[None, :])
            wv_r = wqkv_d[:, F_QK:].rearrange("(o p) f -> p o f", p=128)
            for d in range(D_SUB):
                nc.sync.dma_start(wv_sb[:, d, :], wv_r[:, d, :])
            wp_r = wproj_d.rearrange("(o p) f -> p o f", p=128)
            for d in range(D_SUB):
                nc.sync.dma_start(wproj_sb[:, d, :], wp_r[:, d, :])

            nc.vector.memset(ones_f, 1.0)
            nc.vector.tensor_copy(bqk_col, bqk_col_r)
            # PE warmup: dummy matmuls on the ones tile so the HAM clock
            # gate opens (~3.4us of activity) while inputs are still in
            # flight; real matmuls then start at 2.4GHz.
            for w in range(10):
                psw = big.tile([128, 512], F32, tag="big", name=f"warm_{w}")
                nc.tensor.matmul(
                    psw,
                    lhsT=ones_f[:, 0:128],
                    rhs=ones_f,
                    start=True,
                    stop=True,
                )
            nc.vector.tensor_copy(ones1, ones_f[0:1, :])
            nc.vector.tensor_copy(ones16, ones_f[:, 0:96])
            # ones columns of [v | 1] slots
            v_ones_view = v_sb.rearrange("p s (h c) -> p s h c", c=65)[:, :, :, 64]
            nc.vector.tensor_copy(
                v_ones_view, ones16.rearrange("p (s h) -> p s h", s=8)
            )

            qk_tiles = {}

            # ---- qk^T: one 128-feature tile (f in 0..11), fp16 out;
            # bias folded into the DVE convert.  Emitted as two 512-col
            # halves so the transient PSUM hold (big pool) stays short ----
            def emit_qk_half(f, qh):
                c0 = f * 128
                sl = slice(qh * 512, (qh + 1) * 512)
                psq = big.tile([128, 512], F32, tag="big", name=f"psq_{f}_{qh}")
                for d in range(D_SUB):
                    nc.tensor.matmul(
                        psq,
                        lhsT=wqk_sb[:, d, c0 : c0 + 128],
                        rhs=xT_sb[:, d, sl],
                        start=(d == 0),
                        stop=(d == D_SUB - 1),
                    )
                if qh == 0:
                    qk_tiles[f] = qkT_pool.tile(
                        [128, N], FP16, tag="qkT", name=f"qkT_{f}"
                    )
                nc.vector.tensor_scalar_add(
                    qk_tiles[f][:, sl], psq, bqk_col[:, f : f + 1]
                )

            def emit_qk_tile(f):
                emit_qk_half(f, 0)
                emit_qk_half(f, 1)

            # ---- v m-tile: natural layout, scattered into 65-slots ----
            def emit_v_tile(m):
                psv = big.tile([128, N], F32, tag="big", name=f"psv_{m}")
                for n0, nsz in ((0, 512), (512, 256)):
                    sl = slice(n0, n0 + nsz)
                    for d in range(D_SUB):
                        nc.tensor.matmul(
                            psv[:, sl],
                            lhsT=xT_sb[:, d, m * 128 : (m + 1) * 128],
                            rhs=wv_sb[:, d, sl],
                            start=(d == 0),
                            stop=False,
                        )
                    nc.tensor.matmul(
                        psv[:, sl],
                        lhsT=ones1[0:1, 0:128],
                        rhs=bv_sb[0:1, sl],
                        start=False,
                        stop=True,
                    )
                nc.vector.tensor_copy(
                    v_sb[:, m, :].rearrange("p (h c) -> p h c", c=65)[:, :, 0:64],
                    psv[:, 0:D].rearrange("p (h c) -> p h c", c=64),
                )

            # ---- attention round machinery ----
            def emit_scores(p, kt, i, at_live):
                qT = qk_tiles[p]
                kT = qk_tiles[6 + p]
                pb = slice(64 * i, 64 * i + 64)
                pss = big.tile([128, N], F32, tag="big", name=f"pss_{p}_{kt}_{i}")
                for qh in range(2):
                    sl = slice(qh * 512, (qh + 1) * 512)
                    nc.tensor.matmul(
                        pss[:, sl],
                        lhsT=kT[pb, kt * 128 : (kt + 1) * 128],
                        rhs=qT[pb, sl],
                        start=True,
                        stop=True,
                    )
                at = attnT_pool.tile([128, N], FP16, tag="attnT", name=f"at_{p}_{kt}_{i}")
                nc.scalar.activation(at, pss, func=EXP, scale=SCALE)
                at_live[(kt, i)] = at

            def emit_attnv(p, kt, at_live, pso):
                for i in range(2):
                    h = 2 * p + i
                    at = at_live[(kt, i)]
                    for qh in range(2):
                        nc.tensor.matmul(
                            pso[(i, qh)],
                            lhsT=v_sb[:, kt, h * 65 : h * 65 + 65],
                            rhs=at[:, qh * 512 : (qh + 1) * 512],
                            start=(kt == 0),
                            stop=(kt == TOK_TILES - 1),
                        )

            def emit_norm(p, pso, aoT_sb):
                # phase 1: drain all four accumulators (frees PSUM fast)
                aoUs = {}
                for i in range(2):
                    h = 2 * p + i
                    aoU = aoU_pool.tile([65, N], F32, tag="aoU", name=f"aoU_{h}")
                    for qh in range(2):
                        nc.vector.tensor_copy(
                            aoU[:, qh * 512 : (qh + 1) * 512], pso[(i, qh)]
                        )
                    aoUs[i] = aoU
                # phase 2: den broadcast via DRAM bounce (partition-step-0
                # read is legal from DRAM; gpsimd partition_broadcast gives
                # wrong results on HW), then reciprocal + normalize
                import concourse.bass as bass

                for i in range(2):
                    h = 2 * p + i
                    aoU = aoUs[i]
                    dend = dram_pool.tile([1, N], F32, tag="dend", name=f"dend_{h}")
                    nc.sync.dma_start(dend, aoU[64:65, :])
                    denb = rb_pool.tile([64, N], F32, tag="rb", name=f"denb_{h}")
                    dend_bcast = bass.AP(
                        tensor=dend.tensor,
                        offset=dend.offset,
                        ap=[[0, 64]] + list(dend.ap[1:]),
                    )
                    nc.sync.dma_start(denb, dend_bcast)
                    rbt = rrow_pool.tile([64, N], F32, tag="rrow", name=f"rr_{h}")
                    nc.vector.reciprocal_approx_fast(out=rbt, in_=denb)
                    nc.vector.tensor_tensor(
                        aoT_sb[64 * i : 64 * i + 64, p, :], aoU[0:64, :], rbt, MULT
                    )

            aoT_sb = singles.tile([128, D_SUB, N], BF16)  # 12KB/part

            # ---- prologue: all 12 qk tiles (chases the wqk DMA stream;
            # rounds then run cleanly ACT-paced with no PSUM contention) ----
            for f in (0, 6, 1, 7, 2, 8, 3, 9, 4, 10, 5, 11):
                emit_qk_tile(f)

            # ---- rounds over head pairs ----
            for r in range(6):
                pso = {
                    (i, qh): acc.tile(
                        [65, 512], F32, tag="acc", name=f"pso_{r}_{i}_{qh}"
                    )
                    for i in range(2)
                    for qh in range(2)
                }
                at_live = {}
                for kt in range(TOK_TILES + 1):
                    if kt < TOK_TILES:
                        for i in range(2):
                            emit_scores(r, kt, i, at_live)
                    if r == 0 and kt < TOK_TILES:
                        emit_v_tile(kt)
                    if kt >= 1:
                        emit_attnv(r, kt - 1, at_live, pso)
                emit_norm(r, pso, aoT_sb)

            # ---- output projection.  d<5 partials for 4 m-tiles run while
            # the last round's norm chain (DMA bounce) completes; the d=5
            # finish then streams.  Even m uses the big pool, odd m uses the
            # acc pool (free after round 5) as two column-half tiles so 4
            # tiles fit in PSUM at once.  Bias rides the partial (start). ----
            psy_live = {}
            PROJ_SLICES = ((0, 512), (512, 256))

            def emit_proj_partial(m):
                if m % 2 == 0:
                    psy = big.tile([128, N], F32, tag="big", name=f"psy_{m}")
                    parts = {sl: psy[:, sl[0] : sl[0] + sl[1]] for sl in PROJ_SLICES}
                else:
                    parts = {
                        sl: acc.tile(
                            [128, sl[1]], F32, tag="acc", name=f"psy_{m}_{sl[0]}"
                        )
                        for sl in PROJ_SLICES
                    }
                for n0, nsz in PROJ_SLICES:
                    tgt = parts[(n0, nsz)]
                    sl = slice(n0, n0 + nsz)
                    nc.tensor.matmul(
                        tgt,
                        lhsT=ones1[0:1, 0:128],
                        rhs=bp_sb[0:1, sl],
                        start=True,
                        stop=False,
                    )
                    for d in range(D_SUB - 1):
                        nc.tensor.matmul(
                            tgt,
                            lhsT=aoT_sb[:, d, m * 128 : (m + 1) * 128],
                            rhs=wproj_sb[:, d, sl],
                            start=False,
                            stop=False,
                        )
                psy_live[m] = parts

            def emit_proj_finish(m):
                parts = psy_live.pop(m)
                d = D_SUB - 1
                for n0, nsz in PROJ_SLICES:
                    nc.tensor.matmul(
                        parts[(n0, nsz)],
                        lhsT=aoT_sb[:, d, m * 128 : (m + 1) * 128],
                        rhs=wproj_sb[:, d, slice(n0, n0 + nsz)],
                        start=False,
                        stop=True,
                    )
                ysb = y_pool.tile([128, D], F32, tag="ysb", name=f"ysb_{m}")
                for n0, nsz in PROJ_SLICES:
                    nc.vector.tensor_copy(
                        ysb[:, n0 : n0 + nsz], parts[(n0, nsz)]
                    )
                nc.sync.dma_start(y_d[m * 128 : (m + 1) * 128, :], ysb)

            for m in range(4):
                emit_proj_partial(m)
            for m in range(TOK_TILES):
                emit_proj_finish(m)
                if m + 4 < TOK_TILES:
                    emit_proj_partial(m + 4)

    nc.compile()
    return nc


def _in_maps(x, w_qkv, b_qkv, w_proj, b_proj):
    import ml_dtypes

    bf16 = ml_dtypes.bfloat16
    w_qkv = np.ascontiguousarray(np.asarray(w_qkv, dtype=np.float32).astype(bf16))
    b_qkv = np.ascontiguousarray(b_qkv, dtype=np.float32)
    w_proj = np.ascontiguousarray(np.asarray(w_proj, dtype=np.float32).astype(bf16))
    b_proj = np.ascontiguousarray(b_proj, dtype=np.float32)
    maps = []
    for c in range(N_CORES):
        maps.append(
            {
                "xt": np.ascontiguousarray(
                    np.asarray(x[c], dtype=np.float32).T.astype(bf16)
                ),
                "wqkv": w_qkv,
                "bqkv": b_qkv,
                "wproj": w_proj,
                "bproj": b_proj,
            }
        )
    return maps


def kernel(x, w_qkv, b_qkv, w_proj, b_proj):
    global _cached_nc
    if _cached_nc is None:
        _cached_nc = _build()
    from concourse.bass_utils import run_bass_kernel_spmd

    res = run_bass_kernel_spmd(
        _cached_nc,
        _in_maps(x, w_qkv, b_qkv, w_proj, b_proj),
        list(range(N_CORES)),
    )
    return np.stack([res.results[c]["y"] for c in range(N_CORES)]).astype(np.float32)


if __name__ == "__main__":
    rng = np.random.default_rng(0)
    x = rng.standard_normal((B, N, D), dtype=np.float32)
    w_qkv = rng.standard_normal((D, 3 * D), dtype=np.float32) * D**-0.5
    b_qkv = rng.standard_normal(3 * D).astype(np.float32) * 0.01
    w_proj = rng.standard_normal((D, D), dtype=np.float32) * D**-0.5
    b_proj = rng.standard_normal(D).astype(np.float32) * 0.01
    y = kernel(x, w_qkv, b_qkv, w_proj, b_proj)
    print(y.shape, y.dtype)


# revision 25
# speedup vs baseline: 1.1846x; 1.1846x over previous
"""Multi-head attention (B=8, N=1024, D=768, H=12) on 8 TRN2 NeuronCores.

Sharding: pure data parallel over batch — each core handles one batch
element; weights are replicated. No collectives.

Per-core dataflow (v2 — restructured from the 285us baseline for PE/ACT
overlap and to keep the PE HAM clock warm):

  1. qk^T tiles [128 feat, 1024 tok]: accumulated from block-resident
     wqk (6 DMA blocks of 6KB/partition) x xT in f32r; the qkv BIAS is
     folded into the PSUM->fp16 convert as a DVE tensor_scalar_add with
     a per-partition bias column (saves 2 PE matmuls per tile).
  2. scores^T per (pair, kt, head): ONE fp16 matmul [64K, 128M, 1024N]
     (fp16 moving operand max is 1024) into a [128, 1024] PSUM tile
     from a 2-buf pool -> exp(kt) on ACT overlaps scores(kt+1) on PE.
  3. softmax without max-subtraction (scores ~ N(0,1)); exp scale=1/8.
  4. attn@v: psum[0:65, qh*512] += [v_h | ones]^T @ attnT — 4 separate
     1-bank accumulators per round (i x qh) so pss can double-buffer.
  5. normalize: accumulators drained to SBUF immediately (frees PSUM),
     DVE reciprocal of the den row, gpsimd partition_broadcast (attn
     ucode library) to 64 rows, DVE multiply into f32r attn-out^T.
  6. proj: y = aoT^T @ w_proj + bias (K=1 matmul), DMA out per m-tile.

Rounds are per head-pair; attn@v trails scores by one kt so the PE
never waits on ACT. qk^T tiles for pair r+2 are produced as a burst
inside round r. v-projection tiles are woven into round 0.
"""

import sys

sys.path.insert(0, "/opt/trn_rl_repo")

import numpy as np

B, N, D, H, HD = 8, 1024, 768, 12, 64
F_QK = 2 * D  # 1536
SCALE = HD**-0.5
TOK_TILES = N // 128  # 8
D_SUB = D // 128  # 6
N_CORES = 8

_cached_nc = None


def _build():
    import concourse.tile as tile
    from concourse import bacc, library_config, mybir

    F32 = mybir.dt.float32
    F32R = mybir.dt.float32r
    FP16 = mybir.dt.float16
    BF16 = mybir.dt.bfloat16
    EXP = mybir.ActivationFunctionType.Exp
    MULT = mybir.AluOpType.mult

    nc = bacc.Bacc("TRN2", target_bir_lowering=False, debug=False)

    # x and the big weights ship as bf16 (halves HBM traffic; rel err
    # ~4e-3 vs the 2e-2 gate). Biases stay f32.
    xt_d = nc.dram_tensor("xt", [D, N], BF16, kind="ExternalInput").ap()
    wqkv_d = nc.dram_tensor("wqkv", [D, 3 * D], BF16, kind="ExternalInput").ap()
    bqkv_d = nc.dram_tensor("bqkv", [3 * D], F32R, kind="ExternalInput").ap()
    wproj_d = nc.dram_tensor("wproj", [D, D], BF16, kind="ExternalInput").ap()
    bproj_d = nc.dram_tensor("bproj", [D], F32R, kind="ExternalInput").ap()
    y_d = nc.dram_tensor("y", [N, D], F32, kind="ExternalOutput").ap()

    with tile.TileContext(nc) as tc:
        with (
            # attnT first: ScalarE writes to low SBUF addresses are ~20%
            # faster, and exp (96x [128,1024]) is the ACT bottleneck.
            tc.tile_pool(name="attnT", bufs=4) as attnT_pool,
            tc.tile_pool(name="singles", bufs=1) as singles,
            tc.tile_pool(name="qkT", bufs=12) as qkT_pool,
            tc.tile_pool(name="aoU", bufs=3) as aoU_pool,
            tc.tile_pool(name="rrow", bufs=2) as rrow_pool,
            tc.tile_pool(name="rb", bufs=2) as rb_pool,
            tc.tile_pool(name="dram", bufs=2, space="DRAM") as dram_pool,
            tc.tile_pool(name="yout", bufs=3) as y_pool,
            tc.tile_pool(name="big", bufs=2, space="PSUM") as big,
            tc.tile_pool(name="acc", bufs=4, space="PSUM") as acc,
        ):
            # ---- resident SBUF tensors ----
            xT_sb = singles.tile([128, D_SUB, N], BF16)  # 12KB/part
            wqk_sb = singles.tile([128, D_SUB, F_QK], BF16)  # 18KB/part
            wv_sb = singles.tile([128, D_SUB, D], BF16)  # 9KB/part
            wproj_sb = singles.tile([128, D_SUB, D], BF16)  # 9KB/part
            v_sb = singles.tile([128, TOK_TILES, H * 65], FP16)  # 12.2KB/part
            bqk_col_r = singles.tile([128, 12], F32R)
            bqk_col = singles.tile([128, 12], F32)
            bv_sb = singles.tile([1, D], F32R)
            bp_sb = singles.tile([1, D], F32R)
            ones1 = singles.tile([1, 512], F32R)
            ones16 = singles.tile([128, 96], FP16)
            ones_f = singles.tile([128, 512], F32)

            # ---- setup DMAs, latency-critical order: x and wqk blocks
            # interleaved per d so qk matmuls can chase the stream ----
            xt_r = xt_d.rearrange("(o p) n -> p o n", p=128)
            wqk_r = wqkv_d[:, 0:F_QK].rearrange("(o p) f -> p o f", p=128)
            for d in range(D_SUB):
                nc.sync.dma_start(xT_sb[:, d, :], xt_r[:, d, :])
                nc.sync.dma_start(wqk_sb[:, d, :], wqk_r[:, d, :])
            # per-partition bias column layout: bqk_col[p, f] = bqkv[f*128+p]
            nc.sync.dma_start(
                bqk_col_r, bqkv_d[0:F_QK].rearrange("(f p) -> p f", p=128)
            )
            nc.sync.dma_start(bv_sb, bqkv_d[None, F_QK : 3 * D])
            nc.sync.dma_start(bp_sb, bproj_d[None, :])
            wv_r = wqkv_d[:, F_QK:].rearrange("(o p) f -> p o f", p=128)
            for d in range(D_SUB):
                nc.sync.dma_start(wv_sb[:, d, :], wv_r[:, d, :])
            wp_r = wproj_d.rearrange("(o p) f -> p o f", p=128)
            for d in range(D_SUB):
                nc.sync.dma_start(wproj_sb[:, d, :], wp_r[:, d, :])

            nc.vector.memset(ones_f, 1.0)
            nc.vector.tensor_copy(bqk_col, bqk_col_r)
            # PE warmup: dummy matmuls on the ones tile so the HAM clock
            # gate opens (~3.4us of activity) while inputs are still in
            # flight; real matmuls then start at 2.4GHz.
            for w in range(10):
                psw = big.tile([128, 512], F32, tag="big", name=f"warm_{w}")
                nc.tensor.matmul(
                    psw,
                    lhsT=ones_f[:, 0:128],
                    rhs=ones_f,
                    start=True,
                    stop=True,
                )
            nc.vector.tensor_copy(ones1, ones_f[0:1, :])
            nc.vector.tensor_copy(ones16, ones_f[:, 0:96])
            # ones columns of [v | 1] slots
            v_ones_view = v_sb.rearrange("p s (h c) -> p s h c", c=65)[:, :, :, 64]
            nc.vector.tensor_copy(
                v_ones_view, ones16.rearrange("p (s h) -> p s h", s=8)
            )

            qk_tiles = {}

            # ---- qk^T: one 128-feature tile (f in 0..11), fp16 out;
            # bias folded into the DVE convert.  Emitted as two 512-col
            # halves so the transient PSUM hold (big pool) stays short ----
            def emit_qk_half(f, qh):
                c0 = f * 128
                sl = slice(qh * 512, (qh + 1) * 512)
                psq = big.tile([128, 512], F32, tag="big", name=f"psq_{f}_{qh}")
                for d in range(D_SUB):
                    nc.tensor.matmul(
                        psq,
                        lhsT=wqk_sb[:, d, c0 : c0 + 128],
                        rhs=xT_sb[:, d, sl],
                        start=(d == 0),
                        stop=(d == D_SUB - 1),
                    )
                if qh == 0:
                    qk_tiles[f] = qkT_pool.tile(
                        [128, N], FP16, tag="qkT", name=f"qkT_{f}"
                    )
                nc.vector.tensor_scalar_add(
                    qk_tiles[f][:, sl], psq, bqk_col[:, f : f + 1]
                )

            def emit_qk_tile(f):
                emit_qk_half(f, 0)
                emit_qk_half(f, 1)

            # ---- v m-tile: natural layout, scattered into 65-slots ----
            def emit_v_tile(m):
                psv = big.tile([128, N], F32, tag="big", name=f"psv_{m}")
                for n0, nsz in ((0, 512), (512, 256)):
                    sl = slice(n0, n0 + nsz)
                    for d in range(D_SUB):
                        nc.tensor.matmul(
                            psv[:, sl],
                            lhsT=xT_sb[:, d, m * 128 : (m + 1) * 128],
                            rhs=wv_sb[:, d, sl],
                            start=(d == 0),
                            stop=False,
                        )
                    nc.tensor.matmul(
                        psv[:, sl],
                        lhsT=ones1[0:1, 0:128],
                        rhs=bv_sb[0:1, sl],
                        start=False,
                        stop=True,
                    )
                nc.vector.tensor_copy(
                    v_sb[:, m, :].rearrange("p (h c) -> p h c", c=65)[:, :, 0:64],
                    psv[:, 0:D].rearrange("p (h c) -> p h c", c=64),
                )

            # ---- attention round machinery ----
            def emit_scores(p, kt, i, at_live):
                qT = qk_tiles[p]
                kT = qk_tiles[6 + p]
                pb = slice(64 * i, 64 * i + 64)
                pss = big.tile([128, N], F32, tag="big", name=f"pss_{p}_{kt}_{i}")
                for qh in range(2):
                    sl = slice(qh * 512, (qh + 1) * 512)
                    nc.tensor.matmul(
                        pss[:, sl],
                        lhsT=kT[pb, kt * 128 : (kt + 1) * 128],
                        rhs=qT[pb, sl],
                        start=True,
                        stop=True,
                    )
                at = attnT_pool.tile([128, N], FP16, tag="attnT", name=f"at_{p}_{kt}_{i}")
                nc.scalar.activation(at, pss, func=EXP, scale=SCALE)
                at_live[(kt, i)] = at

            def emit_attnv(p, kt, at_live, pso):
                for i in range(2):
                    h = 2 * p + i
                    at = at_live[(kt, i)]
                    for qh in range(2):
                        nc.tensor.matmul(
                            pso[(i, qh)],
                            lhsT=v_sb[:, kt, h * 65 : h * 65 + 65],
                            rhs=at[:, qh * 512 : (qh + 1) * 512],
                            start=(kt == 0),
                            stop=(kt == TOK_TILES - 1),
                        )

            def emit_norm(p, pso, aoT_sb):
                # phase 1: drain all four accumulators (frees PSUM fast)
                aoUs = {}
                for i in range(2):
                    h = 2 * p + i
                    aoU = aoU_pool.tile([65, N], F32, tag="aoU", name=f"aoU_{h}")
                    for qh in range(2):
                        nc.vector.tensor_copy(
                            aoU[:, qh * 512 : (qh + 1) * 512], pso[(i, qh)]
                        )
                    aoUs[i] = aoU
                # phase 2: den broadcast via DRAM bounce (partition-step-0
                # read is legal from DRAM; gpsimd partition_broadcast gives
                # wrong results on HW), then reciprocal + normalize
                import concourse.bass as bass

                for i in range(2):
                    h = 2 * p + i
                    aoU = aoUs[i]
                    dend = dram_pool.tile([1, N], F32, tag="dend", name=f"dend_{h}")
                    nc.sync.dma_start(dend, aoU[64:65, :])
                    denb = rb_pool.tile([64, N], F32, tag="rb", name=f"denb_{h}")
                    dend_bcast = bass.AP(
                        tensor=dend.tensor,
                        offset=dend.offset,
                        ap=[[0, 64]] + list(dend.ap[1:]),
                    )
                    nc.sync.dma_start(denb, dend_bcast)
                    rbt = rrow_pool.tile([64, N], F32, tag="rrow", name=f"rr_{h}")
                    nc.vector.reciprocal_approx_fast(out=rbt, in_=denb)
                    nc.vector.tensor_tensor(
                        aoT_sb[64 * i : 64 * i + 64, p, :], aoU[0:64, :], rbt, MULT
                    )

            aoT_sb = singles.tile([128, D_SUB, N], BF16)  # 12KB/part

            # ---- prologue: qk tiles for pairs 0 and 1; the remaining 8
            # tiles ride rounds 1-4 as half-tile bursts (ACT-slack rounds) ----
            for f in (0, 6, 1, 7):
                emit_qk_tile(f)
            # (tile_f_q, tile_f_k) prefetched inside round r, used at r+1+
            ROUND_PREFETCH = {1: (2, 8), 2: (3, 9), 3: (4, 10), 4: (5, 11)}

            # ---- rounds over head pairs ----
            for r in range(6):
                pso = {
                    (i, qh): acc.tile(
                        [65, 512], F32, tag="acc", name=f"pso_{r}_{i}_{qh}"
                    )
                    for i in range(2)
                    for qh in range(2)
                }
                at_live = {}
                pf = ROUND_PREFETCH.get(r)
                for kt in range(TOK_TILES + 1):
                    if kt < TOK_TILES:
                        for i in range(2):
                            emit_scores(r, kt, i, at_live)
                    if r == 0 and kt < TOK_TILES:
                        emit_v_tile(kt)
                    if pf is not None and kt in (1, 3, 5, 7):
                        emit_qk_half(pf[kt // 4], (kt // 2) % 2)
                    if kt >= 1:
                        emit_attnv(r, kt - 1, at_live, pso)
                emit_norm(r, pso, aoT_sb)

            # ---- output projection.  d<5 partials for 4 m-tiles run while
            # the last round's norm chain (DMA bounce) completes; the d=5
            # finish then streams.  Even m uses the big pool, odd m uses the
            # acc pool (free after round 5) as two column-half tiles so 4
            # tiles fit in PSUM at once.  Bias rides the partial (start). ----
            psy_live = {}
            PROJ_SLICES = ((0, 512), (512, 256))

            def emit_proj_partial(m):
                if m % 2 == 0:
                    psy = big.tile([128, N], F32, tag="big", name=f"psy_{m}")
                    parts = {sl: psy[:, sl[0] : sl[0] + sl[1]] for sl in PROJ_SLICES}
                else:
                    parts = {
                        sl: acc.tile(
                            [128, sl[1]], F32, tag="acc", name=f"psy_{m}_{sl[0]}"
                        )
                        for sl in PROJ_SLICES
                    }
                for n0, nsz in PROJ_SLICES:
                    tgt = parts[(n0, nsz)]
                    sl = slice(n0, n0 + nsz)
                    nc.tensor.matmul(
                        tgt,
                        lhsT=ones1[0:1, 0:128],
                        rhs=bp_sb[0:1, sl],
                        start=True,
                        stop=False,
                    )
                    for d in range(D_SUB - 1):
                        nc.tensor.matmul(
                            tgt,
                            lhsT=aoT_sb[:, d, m * 128 : (m + 1) * 128],
                            rhs=wproj_sb[:, d, sl],
                            start=False,
                            stop=False,
                        )
                psy_live[m] = parts

            def emit_proj_finish(m):
                parts = psy_live.pop(m)
                d = D_SUB - 1
                for n0, nsz in PROJ_SLICES:
                    nc.tensor.matmul(
                        parts[(n0, nsz)],
                        lhsT=aoT_sb[:, d, m * 128 : (m + 1) * 128],
                        rhs=wproj_sb[:, d, slice(n0, n0 + nsz)],
                        start=False,
                        stop=True,
                    )
                ysb = y_pool.tile([128, D], F32, tag="ysb", name=f"ysb_{m}")
                for n0, nsz in PROJ_SLICES:
                    nc.vector.tensor_copy(
                        ysb[:, n0 : n0 + nsz], parts[(n0, nsz)]
                    )
                nc.sync.dma_start(y_d[m * 128 : (m + 1) * 128, :], ysb)

            for m in range(4):
                emit_proj_partial(m)
            for m in range(TOK_TILES):
                emit_proj_finish(m)
                if m + 4 < TOK_TILES:
                    emit_proj_partial(m + 4)

    nc.compile()
    return nc


def _in_maps(x, w_qkv, b_qkv, w_proj, b_proj):
    import ml_dtypes

    bf16 = ml_dtypes.bfloat16
    w_qkv = np.ascontiguousarray(np.asarray(w_qkv, dtype=np.float32).astype(bf16))
    b_qkv = np.ascontiguousarray(b_qkv, dtype=np.float32)
    w_proj = np.ascontiguousarray(np.asarray(w_proj, dtype=np.float32).astype(bf16))
    b_proj = np.ascontiguousarray(b_proj, dtype=np.float32)
    maps = []
    for c in range(N_CORES):
        maps.append(
            {
                "xt": np.ascontiguousarray(
                    np.asarray(x[c], dtype=np.float32).T.astype(bf16)
                ),
                "wqkv": w_qkv,
                "bqkv": b_qkv,
                "wproj": w_proj,
                "bproj": b_proj,
            }
        )
    return maps


def kernel(x, w_qkv, b_qkv, w_proj, b_proj):
    global _cached_nc
    if _cached_nc is None:
        _cached_nc = _build()
    from concourse.bass_utils import run_bass_kernel_spmd

    res = run_bass_kernel_spmd(
        _cached_nc,
        _in_maps(x, w_qkv, b_qkv, w_proj, b_proj),
        list(range(N_CORES)),
    )
    return np.stack([res.results[c]["y"] for c in range(N_CORES)]).astype(np.float32)


if __name__ == "__main__":
    rng = np.random.default_rng(0)
    x = rng.standard_normal((B, N, D), dtype=np.float32)
    w_qkv = rng.standard_normal((D, 3 * D), dtype=np.float32) * D**-0.5
    b_qkv = rng.standard_normal(3 * D).astype(np.float32) * 0.01
    w_proj = rng.standard_normal((D, D), dtype=np.float32) * D**-0.5
    b_proj = rng.standard_normal(D).astype(np.float32) * 0.01
    y = kernel(x, w_qkv, b_qkv, w_proj, b_proj)
    print(y.shape, y.dtype)


# revision 29
# speedup vs baseline: 1.2108x; 1.0221x over previous
"""Multi-head attention (B=8, N=1024, D=768, H=12) on 8 TRN2 NeuronCores.

Sharding: pure data parallel over batch — each core handles one batch
element; weights are replicated. No collectives.

Per-core dataflow (v2 — restructured from the 285us baseline for PE/ACT
overlap and to keep the PE HAM clock warm):

  1. qk^T tiles [128 feat, 1024 tok]: accumulated from block-resident
     wqk (6 DMA blocks of 6KB/partition) x xT in f32r; the qkv BIAS is
     folded into the PSUM->fp16 convert as a DVE tensor_scalar_add with
     a per-partition bias column (saves 2 PE matmuls per tile).
  2. scores^T per (pair, kt, head): ONE fp16 matmul [64K, 128M, 1024N]
     (fp16 moving operand max is 1024) into a [128, 1024] PSUM tile
     from a 2-buf pool -> exp(kt) on ACT overlaps scores(kt+1) on PE.
  3. softmax without max-subtraction (scores ~ N(0,1)); exp scale=1/8.
  4. attn@v: psum[0:65, qh*512] += [v_h | ones]^T @ attnT — 4 separate
     1-bank accumulators per round (i x qh) so pss can double-buffer.
  5. normalize: accumulators drained to SBUF immediately (frees PSUM),
     DVE reciprocal of the den row, gpsimd partition_broadcast (attn
     ucode library) to 64 rows, DVE multiply into f32r attn-out^T.
  6. proj: y = aoT^T @ w_proj + bias (K=1 matmul), DMA out per m-tile.

Rounds are per head-pair; attn@v trails scores by one kt so the PE
never waits on ACT. qk^T tiles for pair r+2 are produced as a burst
inside round r. v-projection tiles are woven into round 0.
"""

import sys

sys.path.insert(0, "/opt/trn_rl_repo")

import numpy as np

B, N, D, H, HD = 8, 1024, 768, 12, 64
F_QK = 2 * D  # 1536
SCALE = HD**-0.5
TOK_TILES = N // 128  # 8
D_SUB = D // 128  # 6
N_CORES = 8

_cached_nc = None


def _build():
    import concourse.tile as tile
    from concourse import bacc, library_config, mybir

    F32 = mybir.dt.float32
    F32R = mybir.dt.float32r
    FP16 = mybir.dt.float16
    BF16 = mybir.dt.bfloat16
    EXP = mybir.ActivationFunctionType.Exp
    MULT = mybir.AluOpType.mult

    nc = bacc.Bacc("TRN2", target_bir_lowering=False, debug=False)

    # x and the big weights ship as bf16 (halves HBM traffic; rel err
    # ~4e-3 vs the 2e-2 gate). Biases stay f32.
    xt_d = nc.dram_tensor("xt", [D, N], BF16, kind="ExternalInput").ap()
    wqkv_d = nc.dram_tensor("wqkv", [D, 3 * D], BF16, kind="ExternalInput").ap()
    bqkv_d = nc.dram_tensor("bqkv", [3 * D], F32R, kind="ExternalInput").ap()
    wproj_d = nc.dram_tensor("wproj", [D, D], BF16, kind="ExternalInput").ap()
    bproj_d = nc.dram_tensor("bproj", [D], F32R, kind="ExternalInput").ap()
    y_d = nc.dram_tensor("y", [N, D], F32, kind="ExternalOutput").ap()

    with tile.TileContext(nc) as tc:
        with (
            # attnT first: ScalarE writes to low SBUF addresses are ~20%
            # faster, and exp (96x [128,1024]) is the ACT bottleneck.
            tc.tile_pool(name="attnT", bufs=4) as attnT_pool,
            tc.tile_pool(name="singles", bufs=1) as singles,
            tc.tile_pool(name="qkT", bufs=12) as qkT_pool,
            tc.tile_pool(name="aoU", bufs=3) as aoU_pool,
            tc.tile_pool(name="rrow", bufs=2) as rrow_pool,
            tc.tile_pool(name="rb", bufs=2) as rb_pool,
            tc.tile_pool(name="dram", bufs=2, space="DRAM") as dram_pool,
            tc.tile_pool(name="yout", bufs=3) as y_pool,
            tc.tile_pool(name="big", bufs=2, space="PSUM") as big,
            tc.tile_pool(name="acc", bufs=4, space="PSUM") as acc,
        ):
            # ---- resident SBUF tensors ----
            xT_sb = singles.tile([128, D_SUB, N], BF16)  # 12KB/part
            wqk_sb = singles.tile([128, D_SUB, F_QK], BF16)  # 18KB/part
            wv_sb = singles.tile([128, D_SUB, D], BF16)  # 9KB/part
            wproj_sb = singles.tile([128, D_SUB, D], BF16)  # 9KB/part
            v_sb = singles.tile([128, TOK_TILES, H * 65], FP16)  # 12.2KB/part
            bqk_col_r = singles.tile([128, 12], F32R)
            bqk_col = singles.tile([128, 12], F32)
            bv_sb = singles.tile([1, D], F32R)
            bp_sb = singles.tile([1, D], F32R)
            ones1 = singles.tile([1, 512], F32R)
            ones16 = singles.tile([128, 96], FP16)
            ones_f = singles.tile([128, 512], F32)

            # ---- setup DMAs, latency-critical order: x and wqk blocks
            # interleaved per d so qk matmuls can chase the stream ----
            xt_r = xt_d.rearrange("(o p) n -> p o n", p=128)
            wv_r = wqkv_d[:, F_QK:].rearrange("(o p) f -> p o f", p=128)
            wqk_r = wqkv_d[:, 0:F_QK].rearrange("(o p) f -> p o f", p=128)
            for d in range(D_SUB):
                nc.sync.dma_start(xT_sb[:, d, :], xt_r[:, d, :])
                nc.sync.dma_start(wv_sb[:, d, :], wv_r[:, d, :])
            nc.sync.dma_start(bv_sb, bqkv_d[None, F_QK : 3 * D])
            for d in range(D_SUB):
                nc.sync.dma_start(wqk_sb[:, d, :], wqk_r[:, d, :])
            # per-partition bias column layout: bqk_col[p, f] = bqkv[f*128+p]
            nc.sync.dma_start(
                bqk_col_r, bqkv_d[0:F_QK].rearrange("(f p) -> p f", p=128)
            )
            nc.sync.dma_start(bp_sb, bproj_d[None, :])
            wp_r = wproj_d.rearrange("(o p) f -> p o f", p=128)
            for d in range(D_SUB):
                nc.sync.dma_start(wproj_sb[:, d, :], wp_r[:, d, :])

            nc.vector.memset(ones_f, 1.0)
            nc.vector.tensor_copy(bqk_col, bqk_col_r)
            # PE warmup: dummy matmuls on the ones tile so the HAM clock
            # gate opens (~3.4us of activity) while inputs are still in
            # flight; real matmuls then start at 2.4GHz.
            for w in range(10):
                psw = big.tile([128, 512], F32, tag="big", name=f"warm_{w}")
                nc.tensor.matmul(
                    psw,
                    lhsT=ones_f[:, 0:128],
                    rhs=ones_f,
                    start=True,
                    stop=True,
                )
            nc.vector.tensor_copy(ones1, ones_f[0:1, :])
            nc.vector.tensor_copy(ones16, ones_f[:, 0:96])
            # ones columns of [v | 1] slots
            v_ones_view = v_sb.rearrange("p s (h c) -> p s h c", c=65)[:, :, :, 64]
            nc.vector.tensor_copy(
                v_ones_view, ones16.rearrange("p (s h) -> p s h", s=8)
            )

            qk_tiles = {}

            # ---- qk^T: one 128-feature tile (f in 0..11), fp16 out;
            # bias folded into the DVE convert.  Emitted as two 512-col
            # halves so the transient PSUM hold (big pool) stays short ----
            def emit_qk_half(f, qh):
                c0 = f * 128
                sl = slice(qh * 512, (qh + 1) * 512)
                psq = big.tile([128, 512], F32, tag="big", name=f"psq_{f}_{qh}")
                for d in range(D_SUB):
                    nc.tensor.matmul(
                        psq,
                        lhsT=wqk_sb[:, d, c0 : c0 + 128],
                        rhs=xT_sb[:, d, sl],
                        start=(d == 0),
                        stop=(d == D_SUB - 1),
                    )
                if qh == 0:
                    qk_tiles[f] = qkT_pool.tile(
                        [128, N], FP16, tag="qkT", name=f"qkT_{f}"
                    )
                nc.vector.tensor_scalar_add(
                    qk_tiles[f][:, sl], psq, bqk_col[:, f : f + 1]
                )

            def emit_qk_tile(f):
                emit_qk_half(f, 0)
                emit_qk_half(f, 1)

            # ---- v m-tile: natural layout, scattered into 65-slots ----
            def emit_v_tile(m):
                # d-major so each xT stationary tile serves both column
                # slices (one LDWEIGHTS per d instead of two)
                psv = big.tile([128, N], F32, tag="big", name=f"psv_{m}")
                for d in range(D_SUB):
                    for n0, nsz in ((0, 512), (512, 256)):
                        nc.tensor.matmul(
                            psv[:, n0 : n0 + nsz],
                            lhsT=xT_sb[:, d, m * 128 : (m + 1) * 128],
                            rhs=wv_sb[:, d, n0 : n0 + nsz],
                            start=(d == 0),
                            stop=False,
                        )
                for n0, nsz in ((0, 512), (512, 256)):
                    nc.tensor.matmul(
                        psv[:, n0 : n0 + nsz],
                        lhsT=ones1[0:1, 0:128],
                        rhs=bv_sb[0:1, n0 : n0 + nsz],
                        start=False,
                        stop=True,
                    )
                nc.vector.tensor_copy(
                    v_sb[:, m, :].rearrange("p (h c) -> p h c", c=65)[:, :, 0:64],
                    psv[:, 0:D].rearrange("p (h c) -> p h c", c=64),
                )

            # ---- attention round machinery ----
            def emit_scores(p, kt, i, at_live):
                qT = qk_tiles[p]
                kT = qk_tiles[6 + p]
                pb = slice(64 * i, 64 * i + 64)
                pss = big.tile([128, N], F32, tag="big", name=f"pss_{p}_{kt}_{i}")
                for qh in range(2):
                    sl = slice(qh * 512, (qh + 1) * 512)
                    nc.tensor.matmul(
                        pss[:, sl],
                        lhsT=kT[pb, kt * 128 : (kt + 1) * 128],
                        rhs=qT[pb, sl],
                        start=True,
                        stop=True,
                    )
                at = attnT_pool.tile([128, N], FP16, tag="attnT", name=f"at_{p}_{kt}_{i}")
                nc.scalar.activation(at, pss, func=EXP, scale=SCALE)
                at_live[(kt, i)] = at

            def emit_attnv(p, kt, at_live, pso):
                for i in range(2):
                    h = 2 * p + i
                    at = at_live[(kt, i)]
                    for qh in range(2):
                        nc.tensor.matmul(
                            pso[(i, qh)],
                            lhsT=v_sb[:, kt, h * 65 : h * 65 + 65],
                            rhs=at[:, qh * 512 : (qh + 1) * 512],
                            start=(kt == 0),
                            stop=(kt == TOK_TILES - 1),
                        )

            def emit_norm(p, pso, aoT_sb):
                # phase 1: drain all four accumulators (frees PSUM fast)
                aoUs = {}
                for i in range(2):
                    h = 2 * p + i
                    aoU = aoU_pool.tile([65, N], F32, tag="aoU", name=f"aoU_{h}")
                    for qh in range(2):
                        nc.vector.tensor_copy(
                            aoU[:, qh * 512 : (qh + 1) * 512], pso[(i, qh)]
                        )
                    aoUs[i] = aoU
                # phase 2: den broadcast via DRAM bounce (partition-step-0
                # read is legal from DRAM; gpsimd partition_broadcast gives
                # wrong results on HW), then reciprocal + normalize
                import concourse.bass as bass

                for i in range(2):
                    h = 2 * p + i
                    aoU = aoUs[i]
                    dend = dram_pool.tile([1, N], F32, tag="dend", name=f"dend_{h}")
                    nc.sync.dma_start(dend, aoU[64:65, :])
                    denb = rb_pool.tile([64, N], F32, tag="rb", name=f"denb_{h}")
                    dend_bcast = bass.AP(
                        tensor=dend.tensor,
                        offset=dend.offset,
                        ap=[[0, 64]] + list(dend.ap[1:]),
                    )
                    nc.sync.dma_start(denb, dend_bcast)
                    rbt = rrow_pool.tile([64, N], F32, tag="rrow", name=f"rr_{h}")
                    nc.vector.reciprocal_approx_fast(out=rbt, in_=denb)
                    nc.vector.tensor_tensor(
                        aoT_sb[64 * i : 64 * i + 64, p, :], aoU[0:64, :], rbt, MULT
                    )

            aoT_sb = singles.tile([128, D_SUB, N], BF16)  # 12KB/part

            # ---- prologue: first half of the v tiles (x+wv land first),
            # then qk tiles for pairs 0 and 1.  The remaining 8 qk tiles
            # ride rounds 1-4 as half-tile bursts (ACT-slack rounds) ----
            for m in range(4):
                emit_v_tile(m)
            for f in (0, 6, 1, 7):
                emit_qk_tile(f)
            # (tile_f_q, tile_f_k) prefetched inside round r, used at r+1+
            ROUND_PREFETCH = {1: (2, 8), 2: (3, 9), 3: (4, 10), 4: (5, 11)}

            # ---- rounds over head pairs.  Each round's trailing attn@v
            # (kt=7) and norm are hoisted past the next round's first
            # scores so the PE never serializes on the last exp ----
            pending = None
            for r in range(6):
                pso = {
                    (i, qh): acc.tile(
                        [65, 512], F32, tag="acc", name=f"pso_{r}_{i}_{qh}"
                    )
                    for i in range(2)
                    for qh in range(2)
                }
                at_live = {}
                pf = ROUND_PREFETCH.get(r)
                for kt in range(TOK_TILES):
                    for i in range(2):
                        emit_scores(r, kt, i, at_live)
                    if kt == 0 and pending is not None:
                        at_prev, pso_prev, r_prev = pending
                        emit_attnv(r_prev, 7, at_prev, pso_prev)
                        emit_norm(r_prev, pso_prev, aoT_sb)
                        pending = None
                    if r == 0 and kt < 4:
                        emit_v_tile(kt + 4)
                    if pf is not None and kt in (1, 3, 5, 7):
                        emit_qk_half(pf[kt // 4], (kt // 2) % 2)
                    if kt >= 1:
                        emit_attnv(r, kt - 1, at_live, pso)
                if r < 5:
                    pending = (at_live, pso, r)
                else:
                    emit_attnv(r, 7, at_live, pso)
                    emit_norm(r, pso, aoT_sb)

            # ---- output projection.  d<5 partials for 4 m-tiles run while
            # the last round's norm chain (DMA bounce) completes; the d=5
            # finish then streams.  Even m uses the big pool, odd m uses the
            # acc pool (free after round 5) as two column-half tiles so 4
            # tiles fit in PSUM at once.  Bias rides the partial (start). ----
            psy_live = {}
            PROJ_SLICES = ((0, 512), (512, 256))

            def emit_proj_partial(m):
                if m % 2 == 0:
                    psy = big.tile([128, N], F32, tag="big", name=f"psy_{m}")
                    parts = {sl: psy[:, sl[0] : sl[0] + sl[1]] for sl in PROJ_SLICES}
                else:
                    parts = {
                        sl: acc.tile(
                            [128, sl[1]], F32, tag="acc", name=f"psy_{m}_{sl[0]}"
                        )
                        for sl in PROJ_SLICES
                    }
                for n0, nsz in PROJ_SLICES:
                    nc.tensor.matmul(
                        parts[(n0, nsz)],
                        lhsT=ones1[0:1, 0:128],
                        rhs=bp_sb[0:1, n0 : n0 + nsz],
                        start=True,
                        stop=False,
                    )
                for d in range(D_SUB - 1):
                    for n0, nsz in PROJ_SLICES:
                        nc.tensor.matmul(
                            parts[(n0, nsz)],
                            lhsT=aoT_sb[:, d, m * 128 : (m + 1) * 128],
                            rhs=wproj_sb[:, d, n0 : n0 + nsz],
                            start=False,
                            stop=False,
                        )
                psy_live[m] = parts

            def emit_proj_finish(m):
                parts = psy_live.pop(m)
                d = D_SUB - 1
                for n0, nsz in PROJ_SLICES:
                    nc.tensor.matmul(
                        parts[(n0, nsz)],
                        lhsT=aoT_sb[:, d, m * 128 : (m + 1) * 128],
                        rhs=wproj_sb[:, d, slice(n0, n0 + nsz)],
                        start=False,
                        stop=True,
                    )
                ysb = y_pool.tile([128, D], F32, tag="ysb", name=f"ysb_{m}")
                for n0, nsz in PROJ_SLICES:
                    nc.vector.tensor_copy(
                        ysb[:, n0 : n0 + nsz], parts[(n0, nsz)]
                    )
                nc.sync.dma_start(y_d[m * 128 : (m + 1) * 128, :], ysb)

            for m in range(4):
                emit_proj_partial(m)
            for m in range(TOK_TILES):
                emit_proj_finish(m)
                if m + 4 < TOK_TILES:
                    emit_proj_partial(m + 4)

    nc.compile()
    return nc


def _in_maps(x, w_qkv, b_qkv, w_proj, b_proj):
    import ml_dtypes

    bf16 = ml_dtypes.bfloat16
    w_qkv = np.ascontiguousarray(np.asarray(w_qkv, dtype=np.float32).astype(bf16))
    b_qkv = np.ascontiguousarray(b_qkv, dtype=np.float32)
    w_proj = np.ascontiguousarray(np.asarray(w_proj, dtype=np.float32).astype(bf16))
    b_proj = np.ascontiguousarray(b_proj, dtype=np.float32)
    maps = []
    for c in range(N_CORES):
        maps.append(
            {
                "xt": np.ascontiguousarray(
                    np.asarray(x[c], dtype=np.float32).T.astype(bf16)
                ),
                "wqkv": w_qkv,
                "bqkv": b_qkv,
                "wproj": w_proj,
                "bproj": b_proj,
            }
        )
    return maps


def kernel(x, w_qkv, b_qkv, w_proj, b_proj):
    global _cached_nc
    if _cached_nc is None:
        _cached_nc = _build()
    from concourse.bass_utils import run_bass_kernel_spmd

    res = run_bass_kernel_spmd(
        _cached_nc,
        _in_maps(x, w_qkv, b_qkv, w_proj, b_proj),
        list(range(N_CORES)),
    )
    return np.stack([res.results[c]["y"] for c in range(N_CORES)]).astype(np.float32)


if __name__ == "__main__":
    rng = np.random.default_rng(0)
    x = rng.standard_normal((B, N, D), dtype=np.float32)
    w_qkv = rng.standard_normal((D, 3 * D), dtype=np.float32) * D**-0.5
    b_qkv = rng.standard_normal(3 * D).astype(np.float32) * 0.01
    w_proj = rng.standard_normal((D, D), dtype=np.float32) * D**-0.5
    b_proj = rng.standard_normal(D).astype(np.float32) * 0.01
    y = kernel(x, w_qkv, b_qkv, w_proj, b_proj)
    print(y.shape, y.dtype)


# revision 32
# speedup vs baseline: 1.2280x; 1.0142x over previous
"""Multi-head attention (B=8, N=1024, D=768, H=12) on 8 TRN2 NeuronCores.

Sharding: pure data parallel over batch — each core handles one batch
element; weights are replicated. No collectives.

Per-core dataflow (v2 — restructured from the 285us baseline for PE/ACT
overlap and to keep the PE HAM clock warm):

  1. qk^T tiles [128 feat, 1024 tok]: accumulated from block-resident
     wqk (6 DMA blocks of 6KB/partition) x xT in f32r; the qkv BIAS is
     folded into the PSUM->fp16 convert as a DVE tensor_scalar_add with
     a per-partition bias column (saves 2 PE matmuls per tile).
  2. scores^T per (pair, kt, head): ONE fp16 matmul [64K, 128M, 1024N]
     (fp16 moving operand max is 1024) into a [128, 1024] PSUM tile
     from a 2-buf pool -> exp(kt) on ACT overlaps scores(kt+1) on PE.
  3. softmax without max-subtraction (scores ~ N(0,1)); exp scale=1/8.
  4. attn@v: psum[0:65, qh*512] += [v_h | ones]^T @ attnT — 4 separate
     1-bank accumulators per round (i x qh) so pss can double-buffer.
  5. normalize: accumulators drained to SBUF immediately (frees PSUM),
     DVE reciprocal of the den row, gpsimd partition_broadcast (attn
     ucode library) to 64 rows, DVE multiply into f32r attn-out^T.
  6. proj: y = aoT^T @ w_proj + bias (K=1 matmul), DMA out per m-tile.

Rounds are per head-pair; attn@v trails scores by one kt so the PE
never waits on ACT. qk^T tiles for pair r+2 are produced as a burst
inside round r. v-projection tiles are woven into round 0.
"""

import sys

sys.path.insert(0, "/opt/trn_rl_repo")

import numpy as np

B, N, D, H, HD = 8, 1024, 768, 12, 64
F_QK = 2 * D  # 1536
SCALE = HD**-0.5
TOK_TILES = N // 128  # 8
D_SUB = D // 128  # 6
N_CORES = 8

_cached_nc = None


def _build():
    import concourse.tile as tile
    from concourse import bacc, library_config, mybir

    F32 = mybir.dt.float32
    F32R = mybir.dt.float32r
    FP16 = mybir.dt.float16
    BF16 = mybir.dt.bfloat16
    EXP = mybir.ActivationFunctionType.Exp
    MULT = mybir.AluOpType.mult

    nc = bacc.Bacc("TRN2", target_bir_lowering=False, debug=False)

    # x and the big weights ship as bf16 (halves HBM traffic; rel err
    # ~4e-3 vs the 2e-2 gate). Biases stay f32.
    xt_d = nc.dram_tensor("xt", [D, N], BF16, kind="ExternalInput").ap()
    wqkv_d = nc.dram_tensor("wqkv", [D, 3 * D], BF16, kind="ExternalInput").ap()
    bqkv_d = nc.dram_tensor("bqkv", [3 * D], F32R, kind="ExternalInput").ap()
    wproj_d = nc.dram_tensor("wproj", [D, D], BF16, kind="ExternalInput").ap()
    bproj_d = nc.dram_tensor("bproj", [D], F32R, kind="ExternalInput").ap()
    y_d = nc.dram_tensor("y", [N, D], F32, kind="ExternalOutput").ap()

    with tile.TileContext(nc) as tc:
        with (
            # attnT first: ScalarE writes to low SBUF addresses are ~20%
            # faster, and exp (96x [128,1024]) is the ACT bottleneck.
            tc.tile_pool(name="attnT", bufs=6) as attnT_pool,
            tc.tile_pool(name="singles", bufs=1) as singles,
            tc.tile_pool(name="qkT", bufs=12) as qkT_pool,
            tc.tile_pool(name="aoU", bufs=3) as aoU_pool,
            tc.tile_pool(name="rrow", bufs=2) as rrow_pool,
            tc.tile_pool(name="rb", bufs=2) as rb_pool,
            tc.tile_pool(name="dram", bufs=2, space="DRAM") as dram_pool,
            tc.tile_pool(name="yout", bufs=3) as y_pool,
            tc.tile_pool(name="big", bufs=2, space="PSUM") as big,
            tc.tile_pool(name="acc", bufs=4, space="PSUM") as acc,
        ):
            # ---- resident SBUF tensors ----
            xT_sb = singles.tile([128, D_SUB, N], BF16)  # 12KB/part
            wqk_sb = singles.tile([128, D_SUB, F_QK], BF16)  # 18KB/part
            wv_sb = singles.tile([128, D_SUB, D], BF16)  # 9KB/part
            wproj_sb = singles.tile([128, D_SUB, D], BF16)  # 9KB/part
            v_sb = singles.tile([128, TOK_TILES, H * 65], FP16)  # 12.2KB/part
            bqk_col_r = singles.tile([128, 12], F32R)
            bqk_col = singles.tile([128, 12], F32)
            bv_sb = singles.tile([1, D], F32R)
            bp_sb = singles.tile([1, D], F32R)
            ones1 = singles.tile([1, 512], F32R)
            ones16 = singles.tile([128, 96], FP16)
            ones_f = singles.tile([128, 512], F32)

            # ---- setup DMAs, latency-critical order: x and wqk blocks
            # interleaved per d so qk matmuls can chase the stream ----
            xt_r = xt_d.rearrange("(o p) n -> p o n", p=128)
            wv_r = wqkv_d[:, F_QK:].rearrange("(o p) f -> p o f", p=128)
            wqk_r = wqkv_d[:, 0:F_QK].rearrange("(o p) f -> p o f", p=128)
            for d in range(D_SUB):
                nc.sync.dma_start(xT_sb[:, d, :], xt_r[:, d, :])
                nc.sync.dma_start(wv_sb[:, d, :], wv_r[:, d, :])
            nc.sync.dma_start(bv_sb, bqkv_d[None, F_QK : 3 * D])
            for d in range(D_SUB):
                nc.sync.dma_start(wqk_sb[:, d, :], wqk_r[:, d, :])
            # per-partition bias column layout: bqk_col[p, f] = bqkv[f*128+p]
            nc.sync.dma_start(
                bqk_col_r, bqkv_d[0:F_QK].rearrange("(f p) -> p f", p=128)
            )
            nc.sync.dma_start(bp_sb, bproj_d[None, :])
            wp_r = wproj_d.rearrange("(o p) f -> p o f", p=128)
            for d in range(D_SUB):
                nc.sync.dma_start(wproj_sb[:, d, :], wp_r[:, d, :])

            nc.vector.memset(ones_f, 1.0)
            ones_b = singles.tile([128, 512], BF16)
            nc.vector.tensor_copy(ones_b, ones_f)
            nc.vector.tensor_copy(bqk_col, bqk_col_r)
            # PE warmup: bf16 dummy matmuls (single-pass, ~430ns cold each)
            # so the HAM clock gate opens (~3.4us of activity) while inputs
            # are still in flight; real matmuls then start at 2.4GHz.
            for w in range(8):
                psw = big.tile([128, 512], F32, tag="big", name=f"warm_{w}")
                nc.tensor.matmul(
                    psw,
                    lhsT=ones_b[:, 0:128],
                    rhs=ones_b,
                    start=True,
                    stop=True,
                )
            nc.vector.tensor_copy(ones1, ones_f[0:1, :])
            nc.vector.tensor_copy(ones16, ones_f[:, 0:96])
            # ones columns of [v | 1] slots
            v_ones_view = v_sb.rearrange("p s (h c) -> p s h c", c=65)[:, :, :, 64]
            nc.vector.tensor_copy(
                v_ones_view, ones16.rearrange("p (s h) -> p s h", s=8)
            )

            qk_tiles = {}

            # ---- qk^T: one 128-feature tile (f in 0..11), fp16 out;
            # bias folded into the DVE convert.  Emitted as two 512-col
            # halves so the transient PSUM hold (big pool) stays short ----
            def emit_qk_half(f, qh):
                c0 = f * 128
                sl = slice(qh * 512, (qh + 1) * 512)
                psq = big.tile([128, 512], F32, tag="big", name=f"psq_{f}_{qh}")
                for d in range(D_SUB):
                    nc.tensor.matmul(
                        psq,
                        lhsT=wqk_sb[:, d, c0 : c0 + 128],
                        rhs=xT_sb[:, d, sl],
                        start=(d == 0),
                        stop=(d == D_SUB - 1),
                    )
                if qh == 0:
                    qk_tiles[f] = qkT_pool.tile(
                        [128, N], FP16, tag="qkT", name=f"qkT_{f}"
                    )
                nc.vector.tensor_scalar_add(
                    qk_tiles[f][:, sl], psq, bqk_col[:, f : f + 1]
                )

            def emit_qk_tile(f):
                emit_qk_half(f, 0)
                emit_qk_half(f, 1)

            # ---- v m-tile: natural layout, scattered into 65-slots ----
            def emit_v_tile(m):
                # d-major so each xT stationary tile serves both column
                # slices (one LDWEIGHTS per d instead of two)
                psv = big.tile([128, N], F32, tag="big", name=f"psv_{m}")
                for d in range(D_SUB):
                    for n0, nsz in ((0, 512), (512, 256)):
                        nc.tensor.matmul(
                            psv[:, n0 : n0 + nsz],
                            lhsT=xT_sb[:, d, m * 128 : (m + 1) * 128],
                            rhs=wv_sb[:, d, n0 : n0 + nsz],
                            start=(d == 0),
                            stop=False,
                        )
                for n0, nsz in ((0, 512), (512, 256)):
                    nc.tensor.matmul(
                        psv[:, n0 : n0 + nsz],
                        lhsT=ones1[0:1, 0:128],
                        rhs=bv_sb[0:1, n0 : n0 + nsz],
                        start=False,
                        stop=True,
                    )
                nc.vector.tensor_copy(
                    v_sb[:, m, :].rearrange("p (h c) -> p h c", c=65)[:, :, 0:64],
                    psv[:, 0:D].rearrange("p (h c) -> p h c", c=64),
                )

            # ---- attention round machinery ----
            def emit_scores(p, kt, i, at_live):
                qT = qk_tiles[p]
                kT = qk_tiles[6 + p]
                pb = slice(64 * i, 64 * i + 64)
                pss = big.tile([128, N], F32, tag="big", name=f"pss_{p}_{kt}_{i}")
                for qh in range(2):
                    sl = slice(qh * 512, (qh + 1) * 512)
                    nc.tensor.matmul(
                        pss[:, sl],
                        lhsT=kT[pb, kt * 128 : (kt + 1) * 128],
                        rhs=qT[pb, sl],
                        start=True,
                        stop=True,
                    )
                at = attnT_pool.tile([128, N], FP16, tag="attnT", name=f"at_{p}_{kt}_{i}")
                nc.scalar.activation(at, pss, func=EXP, scale=SCALE)
                at_live[(kt, i)] = at

            def emit_attnv(p, kt, at_live, pso):
                for i in range(2):
                    h = 2 * p + i
                    at = at_live[(kt, i)]
                    for qh in range(2):
                        nc.tensor.matmul(
                            pso[(i, qh)],
                            lhsT=v_sb[:, kt, h * 65 : h * 65 + 65],
                            rhs=at[:, qh * 512 : (qh + 1) * 512],
                            start=(kt == 0),
                            stop=(kt == TOK_TILES - 1),
                        )

            def emit_norm(p, pso, aoT_sb):
                # phase 1: drain all four accumulators (frees PSUM fast)
                aoUs = {}
                for i in range(2):
                    h = 2 * p + i
                    aoU = aoU_pool.tile([65, N], F32, tag="aoU", name=f"aoU_{h}")
                    for qh in range(2):
                        nc.vector.tensor_copy(
                            aoU[:, qh * 512 : (qh + 1) * 512], pso[(i, qh)]
                        )
                    aoUs[i] = aoU
                # phase 2: den broadcast via DRAM bounce (partition-step-0
                # read is legal from DRAM; gpsimd partition_broadcast gives
                # wrong results on HW), then reciprocal + normalize
                import concourse.bass as bass

                for i in range(2):
                    h = 2 * p + i
                    aoU = aoUs[i]
                    dend = dram_pool.tile([1, N], F32, tag="dend", name=f"dend_{h}")
                    nc.sync.dma_start(dend, aoU[64:65, :])
                    denb = rb_pool.tile([64, N], F32, tag="rb", name=f"denb_{h}")
                    dend_bcast = bass.AP(
                        tensor=dend.tensor,
                        offset=dend.offset,
                        ap=[[0, 64]] + list(dend.ap[1:]),
                    )
                    nc.sync.dma_start(denb, dend_bcast)
                    rbt = rrow_pool.tile([64, N], F32, tag="rrow", name=f"rr_{h}")
                    nc.vector.reciprocal_approx_fast(out=rbt, in_=denb)
                    nc.vector.tensor_tensor(
                        aoT_sb[64 * i : 64 * i + 64, p, :], aoU[0:64, :], rbt, MULT
                    )

            aoT_sb = singles.tile([128, D_SUB, N], BF16)  # 12KB/part

            # ---- prologue: first half of the v tiles (x+wv land first),
            # then qk tiles for pairs 0 and 1.  The remaining 8 qk tiles
            # ride rounds 1-4 as half-tile bursts (ACT-slack rounds) ----
            for m in range(4):
                emit_v_tile(m)
            for f in (0, 6, 1, 7):
                emit_qk_tile(f)
            # (tile_f_q, tile_f_k) prefetched inside round r, used at r+1+
            ROUND_PREFETCH = {1: (2, 8), 2: (3, 9), 3: (4, 10), 4: (5, 11)}

            # ---- rounds over head pairs.  attn@v trails scores by two kt
            # steps; each round's last two attn@v groups and its norm are
            # hoisted past the next round's first scores so neither the PE
            # nor the accumulator-pool rotation serializes on the last exp ----
            pending = None
            for r in range(6):
                pso = {
                    (i, qh): acc.tile(
                        [65, 512], F32, tag="acc", name=f"pso_{r}_{i}_{qh}"
                    )
                    for i in range(2)
                    for qh in range(2)
                }
                at_live = {}
                pf = ROUND_PREFETCH.get(r)
                for kt in range(TOK_TILES):
                    for i in range(2):
                        emit_scores(r, kt, i, at_live)
                    if kt == 0 and pending is not None:
                        at_prev, pso_prev, r_prev = pending
                        emit_attnv(r_prev, 6, at_prev, pso_prev)
                        emit_attnv(r_prev, 7, at_prev, pso_prev)
                        emit_norm(r_prev, pso_prev, aoT_sb)
                        pending = None
                    if r == 0 and kt < 4:
                        emit_v_tile(kt + 4)
                    if pf is not None and kt in (1, 3, 5, 7):
                        emit_qk_half(pf[kt // 4], (kt // 2) % 2)
                    if kt >= 2:
                        emit_attnv(r, kt - 2, at_live, pso)
                if r < 5:
                    pending = (at_live, pso, r)
                else:
                    emit_attnv(r, 6, at_live, pso)
                    emit_attnv(r, 7, at_live, pso)
                    emit_norm(r, pso, aoT_sb)

            # ---- output projection.  d<5 partials for 4 m-tiles run while
            # the last round's norm chain (DMA bounce) completes; the d=5
            # finish then streams.  Even m uses the big pool, odd m uses the
            # acc pool (free after round 5) as two column-half tiles so 4
            # tiles fit in PSUM at once.  Bias rides the partial (start). ----
            psy_live = {}
            PROJ_SLICES = ((0, 512), (512, 256))

            def emit_proj_partial(m):
                if m % 2 == 0:
                    psy = big.tile([128, N], F32, tag="big", name=f"psy_{m}")
                    parts = {sl: psy[:, sl[0] : sl[0] + sl[1]] for sl in PROJ_SLICES}
                else:
                    parts = {
                        sl: acc.tile(
                            [128, sl[1]], F32, tag="acc", name=f"psy_{m}_{sl[0]}"
                        )
                        for sl in PROJ_SLICES
                    }
                for n0, nsz in PROJ_SLICES:
                    nc.tensor.matmul(
                        parts[(n0, nsz)],
                        lhsT=ones1[0:1, 0:128],
                        rhs=bp_sb[0:1, n0 : n0 + nsz],
                        start=True,
                        stop=False,
                    )
                for d in range(D_SUB - 1):
                    for n0, nsz in PROJ_SLICES:
                        nc.tensor.matmul(
                            parts[(n0, nsz)],
                            lhsT=aoT_sb[:, d, m * 128 : (m + 1) * 128],
                            rhs=wproj_sb[:, d, n0 : n0 + nsz],
                            start=False,
                            stop=False,
                        )
                psy_live[m] = parts

            def emit_proj_finish(m):
                parts = psy_live.pop(m)
                d = D_SUB - 1
                for n0, nsz in PROJ_SLICES:
                    nc.tensor.matmul(
                        parts[(n0, nsz)],
                        lhsT=aoT_sb[:, d, m * 128 : (m + 1) * 128],
                        rhs=wproj_sb[:, d, slice(n0, n0 + nsz)],
                        start=False,
                        stop=True,
                    )
                ysb = y_pool.tile([128, D], F32, tag="ysb", name=f"ysb_{m}")
                for n0, nsz in PROJ_SLICES:
                    nc.vector.tensor_copy(
                        ysb[:, n0 : n0 + nsz], parts[(n0, nsz)]
                    )
                nc.sync.dma_start(y_d[m * 128 : (m + 1) * 128, :], ysb)

            for m in range(4):
                emit_proj_partial(m)
            for m in range(TOK_TILES):
                emit_proj_finish(m)
                if m + 4 < TOK_TILES:
                    emit_proj_partial(m + 4)

    nc.compile()
    return nc


def _in_maps(x, w_qkv, b_qkv, w_proj, b_proj):
    import ml_dtypes

    bf16 = ml_dtypes.bfloat16
    w_qkv = np.ascontiguousarray(np.asarray(w_qkv, dtype=np.float32).astype(bf16))
    b_qkv = np.ascontiguousarray(b_qkv, dtype=np.float32)
    w_proj = np.ascontiguousarray(np.asarray(w_proj, dtype=np.float32).astype(bf16))
    b_proj = np.ascontiguousarray(b_proj, dtype=np.float32)
    maps = []
    for c in range(N_CORES):
        maps.append(
            {
                "xt": np.ascontiguousarray(
                    np.asarray(x[c], dtype=np.float32).T.astype(bf16)
                ),
                "wqkv": w_qkv,
                "bqkv": b_qkv,
                "wproj": w_proj,
                "bproj": b_proj,
            }
        )
    return maps


def kernel(x, w_qkv, b_qkv, w_proj, b_proj):
    global _cached_nc
    if _cached_nc is None:
        _cached_nc = _build()
    from concourse.bass_utils import run_bass_kernel_spmd

    res = run_bass_kernel_spmd(
        _cached_nc,
        _in_maps(x, w_qkv, b_qkv, w_proj, b_proj),
        list(range(N_CORES)),
    )
    return np.stack([res.results[c]["y"] for c in range(N_CORES)]).astype(np.float32)


if __name__ == "__main__":
    rng = np.random.default_rng(0)
    x = rng.standard_normal((B, N, D), dtype=np.float32)
    w_qkv = rng.standard_normal((D, 3 * D), dtype=np.float32) * D**-0.5
    b_qkv = rng.standard_normal(3 * D).astype(np.float32) * 0.01
    w_proj = rng.standard_normal((D, D), dtype=np.float32) * D**-0.5
    b_proj = rng.standard_normal(D).astype(np.float32) * 0.01
    y = kernel(x, w_qkv, b_qkv, w_proj, b_proj)
    print(y.shape, y.dtype)


# revision 34
# speedup vs baseline: 1.2284x; 1.0003x over previous
"""Multi-head attention (B=8, N=1024, D=768, H=12) on 8 TRN2 NeuronCores.

Sharding: pure data parallel over batch — each core handles one batch
element; weights are replicated. No collectives.

Per-core dataflow (v2 — restructured from the 285us baseline for PE/ACT
overlap and to keep the PE HAM clock warm):

  1. qk^T tiles [128 feat, 1024 tok]: accumulated from block-resident
     wqk (6 DMA blocks of 6KB/partition) x xT in f32r; the qkv BIAS is
     folded into the PSUM->fp16 convert as a DVE tensor_scalar_add with
     a per-partition bias column (saves 2 PE matmuls per tile).
  2. scores^T per (pair, kt, head): ONE fp16 matmul [64K, 128M, 1024N]
     (fp16 moving operand max is 1024) into a [128, 1024] PSUM tile
     from a 2-buf pool -> exp(kt) on ACT overlaps scores(kt+1) on PE.
  3. softmax without max-subtraction (scores ~ N(0,1)); exp scale=1/8.
  4. attn@v: psum[0:65, qh*512] += [v_h | ones]^T @ attnT — 4 separate
     1-bank accumulators per round (i x qh) so pss can double-buffer.
  5. normalize: accumulators drained to SBUF immediately (frees PSUM),
     DVE reciprocal of the den row, gpsimd partition_broadcast (attn
     ucode library) to 64 rows, DVE multiply into f32r attn-out^T.
  6. proj: y = aoT^T @ w_proj + bias (K=1 matmul), DMA out per m-tile.

Rounds are per head-pair; attn@v trails scores by one kt so the PE
never waits on ACT. qk^T tiles for pair r+2 are produced as a burst
inside round r. v-projection tiles are woven into round 0.
"""

import sys

sys.path.insert(0, "/opt/trn_rl_repo")

import numpy as np

B, N, D, H, HD = 8, 1024, 768, 12, 64
F_QK = 2 * D  # 1536
SCALE = HD**-0.5
TOK_TILES = N // 128  # 8
D_SUB = D // 128  # 6
N_CORES = 8

_cached_nc = None


def _build():
    import concourse.tile as tile
    from concourse import bacc, library_config, mybir

    F32 = mybir.dt.float32
    F32R = mybir.dt.float32r
    FP16 = mybir.dt.float16
    BF16 = mybir.dt.bfloat16
    EXP = mybir.ActivationFunctionType.Exp
    MULT = mybir.AluOpType.mult

    nc = bacc.Bacc("TRN2", target_bir_lowering=False, debug=False)

    # x and the big weights ship as bf16 (halves HBM traffic; rel err
    # ~4e-3 vs the 2e-2 gate). Biases stay f32.
    xt_d = nc.dram_tensor("xt", [D, N], BF16, kind="ExternalInput").ap()
    wqkv_d = nc.dram_tensor("wqkv", [D, 3 * D], BF16, kind="ExternalInput").ap()
    bqkv_d = nc.dram_tensor("bqkv", [3 * D], F32R, kind="ExternalInput").ap()
    wproj_d = nc.dram_tensor("wproj", [D, D], BF16, kind="ExternalInput").ap()
    bproj_d = nc.dram_tensor("bproj", [D], F32R, kind="ExternalInput").ap()
    y_d = nc.dram_tensor("y", [N, D], F32, kind="ExternalOutput").ap()

    with tile.TileContext(nc) as tc:
        with (
            # attnT first: ScalarE writes to low SBUF addresses are ~20%
            # faster, and exp (96x [128,1024]) is the ACT bottleneck.
            tc.tile_pool(name="attnT", bufs=6) as attnT_pool,
            tc.tile_pool(name="singles", bufs=1) as singles,
            tc.tile_pool(name="qkT", bufs=12) as qkT_pool,
            tc.tile_pool(name="aoU", bufs=3) as aoU_pool,
            tc.tile_pool(name="rrow", bufs=2) as rrow_pool,
            tc.tile_pool(name="rb", bufs=2) as rb_pool,
            tc.tile_pool(name="dram", bufs=2, space="DRAM") as dram_pool,
            tc.tile_pool(name="yout", bufs=3) as y_pool,
            tc.tile_pool(name="big", bufs=2, space="PSUM") as big,
            tc.tile_pool(name="acc", bufs=4, space="PSUM") as acc,
        ):
            # ---- resident SBUF tensors ----
            xT_sb = singles.tile([128, D_SUB, N], BF16)  # 12KB/part
            wqk_sb = singles.tile([128, D_SUB, F_QK], BF16)  # 18KB/part
            wv_sb = singles.tile([128, D_SUB, D], BF16)  # 9KB/part
            wproj_sb = singles.tile([128, D_SUB, D], BF16)  # 9KB/part
            v_sb = singles.tile([128, TOK_TILES, H * 65], FP16)  # 12.2KB/part
            bqk_col_r = singles.tile([128, 12], F32R)
            bqk_col = singles.tile([128, 12], F32)
            bv_sb = singles.tile([1, D], F32R)
            bp_sb = singles.tile([1, D], F32R)
            ones1 = singles.tile([1, 512], F32R)
            ones16 = singles.tile([128, 96], FP16)
            ones_f = singles.tile([128, 512], F32)

            # ---- setup DMAs, latency-critical order: x and wqk blocks
            # interleaved per d so qk matmuls can chase the stream ----
            xt_r = xt_d.rearrange("(o p) n -> p o n", p=128)
            wv_r = wqkv_d[:, F_QK:].rearrange("(o p) f -> p o f", p=128)
            wqk_r = wqkv_d[:, 0:F_QK].rearrange("(o p) f -> p o f", p=128)
            for d in range(D_SUB):
                nc.sync.dma_start(xT_sb[:, d, :], xt_r[:, d, :])
                nc.sync.dma_start(wv_sb[:, d, :], wv_r[:, d, :])
            nc.sync.dma_start(bv_sb, bqkv_d[None, F_QK : 3 * D])
            for d in range(D_SUB):
                nc.sync.dma_start(wqk_sb[:, d, :], wqk_r[:, d, :])
            # per-partition bias column layout: bqk_col[p, f] = bqkv[f*128+p]
            nc.sync.dma_start(
                bqk_col_r, bqkv_d[0:F_QK].rearrange("(f p) -> p f", p=128)
            )
            nc.sync.dma_start(bp_sb, bproj_d[None, :])
            wp_r = wproj_d.rearrange("(o p) f -> p o f", p=128)
            for d in range(D_SUB):
                nc.sync.dma_start(wproj_sb[:, d, :], wp_r[:, d, :])

            nc.vector.memset(ones_f, 1.0)
            ones_b = singles.tile([128, 512], BF16)
            nc.vector.tensor_copy(ones_b, ones_f)
            nc.vector.tensor_copy(bqk_col, bqk_col_r)
            # PE warmup: bf16 dummy matmuls (single-pass, ~430ns cold each)
            # so the HAM clock gate opens (~3.4us of activity) while inputs
            # are still in flight; real matmuls then start at 2.4GHz.
            for w in range(8):
                psw = big.tile([128, 512], F32, tag="big", name=f"warm_{w}")
                nc.tensor.matmul(
                    psw,
                    lhsT=ones_b[:, 0:128],
                    rhs=ones_b,
                    start=True,
                    stop=True,
                )
            nc.vector.tensor_copy(ones1, ones_f[0:1, :])
            nc.vector.tensor_copy(ones16, ones_f[:, 0:96])
            # ones columns of [v | 1] slots
            v_ones_view = v_sb.rearrange("p s (h c) -> p s h c", c=65)[:, :, :, 64]
            nc.vector.tensor_copy(
                v_ones_view, ones16.rearrange("p (s h) -> p s h", s=8)
            )

            qk_tiles = {}

            # ---- qk^T: one 128-feature tile (f in 0..11), fp16 out;
            # bias folded into the DVE convert.  Emitted as two 512-col
            # halves so the transient PSUM hold (big pool) stays short ----
            def emit_qk_half(f, qh):
                c0 = f * 128
                sl = slice(qh * 512, (qh + 1) * 512)
                psq = big.tile([128, 512], F32, tag="big", name=f"psq_{f}_{qh}")
                for d in range(D_SUB):
                    nc.tensor.matmul(
                        psq,
                        lhsT=wqk_sb[:, d, c0 : c0 + 128],
                        rhs=xT_sb[:, d, sl],
                        start=(d == 0),
                        stop=(d == D_SUB - 1),
                    )
                if qh == 0:
                    qk_tiles[f] = qkT_pool.tile(
                        [128, N], FP16, tag="qkT", name=f"qkT_{f}"
                    )
                nc.vector.tensor_scalar_add(
                    qk_tiles[f][:, sl], psq, bqk_col[:, f : f + 1]
                )

            def emit_qk_tile(f):
                emit_qk_half(f, 0)
                emit_qk_half(f, 1)

            # ---- v m-tile: natural layout, scattered into 65-slots ----
            def emit_v_tile(m):
                # d-major so each xT stationary tile serves both column
                # slices (one LDWEIGHTS per d instead of two)
                psv = big.tile([128, N], F32, tag="big", name=f"psv_{m}")
                for d in range(D_SUB):
                    for n0, nsz in ((0, 512), (512, 256)):
                        nc.tensor.matmul(
                            psv[:, n0 : n0 + nsz],
                            lhsT=xT_sb[:, d, m * 128 : (m + 1) * 128],
                            rhs=wv_sb[:, d, n0 : n0 + nsz],
                            start=(d == 0),
                            stop=False,
                        )
                for n0, nsz in ((0, 512), (512, 256)):
                    nc.tensor.matmul(
                        psv[:, n0 : n0 + nsz],
                        lhsT=ones1[0:1, 0:128],
                        rhs=bv_sb[0:1, n0 : n0 + nsz],
                        start=False,
                        stop=True,
                    )
                nc.vector.tensor_copy(
                    v_sb[:, m, :].rearrange("p (h c) -> p h c", c=65)[:, :, 0:64],
                    psv[:, 0:D].rearrange("p (h c) -> p h c", c=64),
                )

            # ---- attention round machinery ----
            def emit_scores(p, kt, i, at_live):
                qT = qk_tiles[p]
                kT = qk_tiles[6 + p]
                pb = slice(64 * i, 64 * i + 64)
                pss = big.tile([128, N], F32, tag="big", name=f"pss_{p}_{kt}_{i}")
                for qh in range(2):
                    sl = slice(qh * 512, (qh + 1) * 512)
                    nc.tensor.matmul(
                        pss[:, sl],
                        lhsT=kT[pb, kt * 128 : (kt + 1) * 128],
                        rhs=qT[pb, sl],
                        start=True,
                        stop=True,
                    )
                at = attnT_pool.tile([128, N], FP16, tag="attnT", name=f"at_{p}_{kt}_{i}")
                nc.scalar.activation(at, pss, func=EXP, scale=SCALE)
                at_live[(kt, i)] = at

            def emit_attnv(p, kt, at_live, pso):
                for i in range(2):
                    h = 2 * p + i
                    at = at_live[(kt, i)]
                    for qh in range(2):
                        nc.tensor.matmul(
                            pso[(i, qh)],
                            lhsT=v_sb[:, kt, h * 65 : h * 65 + 65],
                            rhs=at[:, qh * 512 : (qh + 1) * 512],
                            start=(kt == 0),
                            stop=(kt == TOK_TILES - 1),
                        )

            def emit_norm(p, pso, aoT_sb):
                # phase 1: drain all four accumulators (frees PSUM fast)
                aoUs = {}
                for i in range(2):
                    h = 2 * p + i
                    aoU = aoU_pool.tile([65, N], F32, tag="aoU", name=f"aoU_{h}")
                    for qh in range(2):
                        nc.vector.tensor_copy(
                            aoU[:, qh * 512 : (qh + 1) * 512], pso[(i, qh)]
                        )
                    aoUs[i] = aoU
                # phase 2: den broadcast via DRAM bounce (partition-step-0
                # read is legal from DRAM; gpsimd partition_broadcast gives
                # wrong results on HW), then reciprocal + normalize
                import concourse.bass as bass

                for i in range(2):
                    h = 2 * p + i
                    aoU = aoUs[i]
                    dend = dram_pool.tile([1, N], F32, tag="dend", name=f"dend_{h}")
                    nc.sync.dma_start(dend, aoU[64:65, :])
                    denb = rb_pool.tile([64, N], F32, tag="rb", name=f"denb_{h}")
                    dend_bcast = bass.AP(
                        tensor=dend.tensor,
                        offset=dend.offset,
                        ap=[[0, 64]] + list(dend.ap[1:]),
                    )
                    nc.sync.dma_start(denb, dend_bcast)
                    rbt = rrow_pool.tile([64, N], F32, tag="rrow", name=f"rr_{h}")
                    nc.vector.reciprocal_approx_fast(out=rbt, in_=denb)
                    nc.vector.tensor_tensor(
                        aoT_sb[64 * i : 64 * i + 64, p, :], aoU[0:64, :], rbt, MULT
                    )

            aoT_sb = singles.tile([128, D_SUB, N], BF16)  # 12KB/part

            # ---- prologue: first half of the v tiles (x+wv land first),
            # then qk tiles for pair 0 only.  The remaining 10 qk tiles
            # ride rounds 0-4 as half-tile bursts ----
            for m in range(4):
                emit_v_tile(m)
            for f in (0, 6):
                emit_qk_tile(f)
            # (tile_f_q, tile_f_k) prefetched inside round r, used at r+1+.
            # kt slots avoid kt=7 so the last psq's DVE convert never sits
            # in the pss rotation at the round boundary.
            ROUND_PREFETCH = {
                0: (1, 7),
                1: (2, 8),
                2: (3, 9),
                3: (4, 10),
                4: (5, 11),
            }
            BURST_KTS = {0: (3, 4, 5, 6), 1: (1, 3, 5, 6), 2: (1, 3, 5, 6),
                         3: (1, 3, 5, 6), 4: (1, 3, 5, 6)}

            # ---- rounds over head pairs.  attn@v trails scores by two kt
            # steps; each round's last two attn@v groups and its norm are
            # hoisted past the next round's first scores so neither the PE
            # nor the accumulator-pool rotation serializes on the last exp ----
            pending = None
            for r in range(6):
                pso = {
                    (i, qh): acc.tile(
                        [65, 512], F32, tag="acc", name=f"pso_{r}_{i}_{qh}"
                    )
                    for i in range(2)
                    for qh in range(2)
                }
                at_live = {}
                pf = ROUND_PREFETCH.get(r)
                kts = BURST_KTS.get(r, ())
                for kt in range(TOK_TILES):
                    for i in range(2):
                        emit_scores(r, kt, i, at_live)
                    if kt == 0 and pending is not None:
                        at_prev, pso_prev, r_prev = pending
                        emit_attnv(r_prev, 6, at_prev, pso_prev)
                        emit_attnv(r_prev, 7, at_prev, pso_prev)
                    if r == 0 and kt < 4:
                        emit_v_tile(kt + 4)
                    if pf is not None and kt in kts:
                        j = kts.index(kt)
                        emit_qk_half(pf[j // 2], j % 2)
                    if kt == 1 and pending is not None:
                        at_prev, pso_prev, r_prev = pending
                        emit_norm(r_prev, pso_prev, aoT_sb)
                        pending = None
                    if kt >= 2:
                        emit_attnv(r, kt - 2, at_live, pso)
                if r < 5:
                    pending = (at_live, pso, r)
                else:
                    emit_attnv(r, 6, at_live, pso)
                    emit_attnv(r, 7, at_live, pso)
                    emit_norm(r, pso, aoT_sb)

            # ---- output projection.  d<5 partials for 4 m-tiles run while
            # the last round's norm chain (DMA bounce) completes; the d=5
            # finish then streams.  Even m uses the big pool, odd m uses the
            # acc pool (free after round 5) as two column-half tiles so 4
            # tiles fit in PSUM at once.  Bias rides the partial (start). ----
            psy_live = {}
            PROJ_SLICES = ((0, 512), (512, 256))

            def emit_proj_partial(m):
                if m % 2 == 0:
                    psy = big.tile([128, N], F32, tag="big", name=f"psy_{m}")
                    parts = {sl: psy[:, sl[0] : sl[0] + sl[1]] for sl in PROJ_SLICES}
                else:
                    parts = {
                        sl: acc.tile(
                            [128, sl[1]], F32, tag="acc", name=f"psy_{m}_{sl[0]}"
                        )
                        for sl in PROJ_SLICES
                    }
                for n0, nsz in PROJ_SLICES:
                    nc.tensor.matmul(
                        parts[(n0, nsz)],
                        lhsT=ones1[0:1, 0:128],
                        rhs=bp_sb[0:1, n0 : n0 + nsz],
                        start=True,
                        stop=False,
                    )
                for d in range(D_SUB - 1):
                    for n0, nsz in PROJ_SLICES:
                        nc.tensor.matmul(
                            parts[(n0, nsz)],
                            lhsT=aoT_sb[:, d, m * 128 : (m + 1) * 128],
                            rhs=wproj_sb[:, d, n0 : n0 + nsz],
                            start=False,
                            stop=False,
                        )
                psy_live[m] = parts

            def emit_proj_finish(m):
                parts = psy_live.pop(m)
                d = D_SUB - 1
                for n0, nsz in PROJ_SLICES:
                    nc.tensor.matmul(
                        parts[(n0, nsz)],
                        lhsT=aoT_sb[:, d, m * 128 : (m + 1) * 128],
                        rhs=wproj_sb[:, d, slice(n0, n0 + nsz)],
                        start=False,
                        stop=True,
                    )
                ysb = y_pool.tile([128, D], F32, tag="ysb", name=f"ysb_{m}")
                for n0, nsz in PROJ_SLICES:
                    nc.vector.tensor_copy(
                        ysb[:, n0 : n0 + nsz], parts[(n0, nsz)]
                    )
                nc.sync.dma_start(y_d[m * 128 : (m + 1) * 128, :], ysb)

            for m in range(4):
                emit_proj_partial(m)
            for m in range(TOK_TILES):
                emit_proj_finish(m)
                if m + 4 < TOK_TILES:
                    emit_proj_partial(m + 4)

    nc.compile()
    return nc


def _in_maps(x, w_qkv, b_qkv, w_proj, b_proj):
    import ml_dtypes

    bf16 = ml_dtypes.bfloat16
    w_qkv = np.ascontiguousarray(np.asarray(w_qkv, dtype=np.float32).astype(bf16))
    b_qkv = np.ascontiguousarray(b_qkv, dtype=np.float32)
    w_proj = np.ascontiguousarray(np.asarray(w_proj, dtype=np.float32).astype(bf16))
    b_proj = np.ascontiguousarray(b_proj, dtype=np.float32)
    maps = []
    for c in range(N_CORES):
        maps.append(
            {
                "xt": np.ascontiguousarray(
                    np.asarray(x[c], dtype=np.float32).T.astype(bf16)
                ),
                "wqkv": w_qkv,
                "bqkv": b_qkv,
                "wproj": w_proj,
                "bproj": b_proj,
            }
        )
    return maps


def kernel(x, w_qkv, b_qkv, w_proj, b_proj):
    global _cached_nc
    if _cached_nc is None:
        _cached_nc = _build()
    from concourse.bass_utils import run_bass_kernel_spmd

    res = run_bass_kernel_spmd(
        _cached_nc,
        _in_maps(x, w_qkv, b_qkv, w_proj, b_proj),
        list(range(N_CORES)),
    )
    return np.stack([res.results[c]["y"] for c in range(N_CORES)]).astype(np.float32)


if __name__ == "__main__":
    rng = np.random.default_rng(0)
    x = rng.standard_normal((B, N, D), dtype=np.float32)
    w_qkv = rng.standard_normal((D, 3 * D), dtype=np.float32) * D**-0.5
    b_qkv = rng.standard_normal(3 * D).astype(np.float32) * 0.01
    w_proj = rng.standard_normal((D, D), dtype=np.float32) * D**-0.5
    b_proj = rng.standard_normal(D).astype(np.float32) * 0.01
    y = kernel(x, w_qkv, b_qkv, w_proj, b_proj)
    print(y.shape, y.dtype)
